# revision 1
# baseline (speedup 1.0000x reference)
"""MedianPool2d (3x3, stride 1, zero-pad 1) Trainium2 Bass kernel.

Full input x: (8, 64, 256, 256) fp32.  Sharding: pure data parallel over
batch -> core i processes x[i] (64, 256, 256).

Per-core layout: 128 SBUF partitions = (h, c) with p = h*64 + c, where
h in {0,1} picks the top/bottom 128-row half of the image and c the
channel.  Each partition processes a strip of HH=128 rows x 256 cols,
with a 1-row halo on each side (zero at the image border, neighbor rows
at the half boundary - both come in via DMA / memset).  Rows are stored
padded to WP=258 with zero columns at 0 and 257, so every tap of the
3x3 window is a pure free-dim offset.

Median of 9 = med3(max3(column mins), med3(column medians),
                   min3(column maxes))  -- exact, 15 min/max passes/pixel
with vertical row-pair sharing and horizontal even/odd pair sharing.
All elementwise work runs on the DVE (this toolchain rejects
TensorTensor on GPSIMD and CCE min/max accum on DMA); DMA is fully
overlapped by the Tile scheduler, and merge/final temporaries alias the
slots of dead earlier-stage buffers so R=16 chunks fit in SBUF.
"""

import numpy as np

B, C, H, W = 8, 64, 256, 256
NCORES = 8
HH = H // 2          # rows per half-strip
WP = W + 2           # padded row width

_CACHE = {}


def _build(R=8, gp_rows=0):
    """Build the Bass module for one core: x (64,256,256) f32 -> out same.

    gp_rows: number of output rows (of each chunk's R) computed on GPSIMD
    instead of the DVE.
    """
    import concourse.bacc as bacc
    import concourse.mybir as mybir
    from concourse.tile import TileContext

    MIN = mybir.AluOpType.min
    MAX = mybir.AluOpType.max
    f32 = mybir.dt.float32

    assert HH % R == 0
    assert 0 <= gp_rows < R
    K = HH // R                     # chunks per strip

    nc = bacc.Bacc("TRN2", name="median_pool2d")
    x = nc.dram_tensor("x", [C, H, W], f32, kind="ExternalInput")
    out = nc.dram_tensor("out", [C, H, W], f32, kind="ExternalOutput")

    xg = x.ap()                     # global view [c, 256, 256]
    og = out.ap()

    def tt(out_ap, in0, in1, op):
        """Elementwise tensor_tensor, row-split DVE/GPSIMD.

        All APs are [128, rows, width]; the row dim is axis 1.
        """
        rows = out_ap.shape[1]
        split = rows - gp_rows if rows > gp_rows else rows
        nc.vector.tensor_tensor(
            out=out_ap[:, 0:split], in0=in0[:, 0:split], in1=in1[:, 0:split],
            op=op,
        )
        if split < rows:
            nc.gpsimd.tensor_tensor(
                out=out_ap[:, split:rows], in0=in0[:, split:rows],
                in1=in1[:, split:rows], op=op,
            )

    with TileContext(nc) as tc:
        with (
            tc.tile_pool(name="io_in", bufs=3) as in_pool,
            tc.tile_pool(name="io_out", bufs=3) as out_pool,
            tc.tile_pool(name="vert", bufs=1) as v_pool,
            tc.tile_pool(name="merge", bufs=1) as m_pool,
        ):
            for k in range(K):
                r0 = k * R                      # first output row (half-local)
                # ---- load input chunk: rows r0-1 .. r0+R (R+2 rows) ----
                it = in_pool.tile([128, (R + 2) * WP], f32, name="it", tag="it")
                it3 = it.rearrange("p (r w) -> p r w", w=WP)
                # zero pad columns 0 and 257 for all rows
                nc.vector.memset(it3[:, :, 0:WP:WP - 1], 0.0)
                # top half: global rows r0-1 .. r0+R+1 (clip at k==0)
                if k == 0:
                    nc.vector.memset(it3[0:64, 0:1, 1:W + 1], 0.0)
                    nc.sync.dma_start(
                        out=it3[0:64, 1:R + 2, 1:W + 1],
                        in_=xg[:, 0:R + 1, :],
                    )
                else:
                    nc.sync.dma_start(
                        out=it3[0:64, :, 1:W + 1],
                        in_=xg[:, r0 - 1:r0 + R + 1, :],
                    )
                # bottom half: global rows HH+r0-1 .. HH+r0+R+1 (clip at last)
                if k == K - 1:
                    nc.vector.memset(it3[64:128, R + 1:R + 2, 1:W + 1], 0.0)
                    nc.sync.dma_start(
                        out=it3[64:128, 0:R + 1, 1:W + 1],
                        in_=xg[:, HH + r0 - 1:H, :],
                    )
                else:
                    nc.sync.dma_start(
                        out=it3[64:128, :, 1:W + 1],
                        in_=xg[:, HH + r0 - 1:HH + r0 + R + 1, :],
                    )

                # ---- vertical sort3 over rows (full padded width) ----
                X0 = it3[:, 0:R, :]
                X1 = it3[:, 1:R + 1, :]
                X2 = it3[:, 2:R + 2, :]

                def vtile(name):
                    t = v_pool.tile([128, R * WP], f32, name=name, tag=name)
                    return t.rearrange("p (r w) -> p r w", w=WP)

                P3 = vtile("bP")
                Q3 = vtile("bQ")
                Lo3 = vtile("bLo")
                W3 = vtile("bW")
                Me3 = vtile("bMe")
                Hi3 = vtile("bHi")

                tt(P3, X0, X1, MIN)
                tt(Q3, X0, X1, MAX)
                tt(Lo3, P3, X2, MIN)
                tt(W3, Q3, X2, MIN)
                tt(Me3, P3, W3, MAX)
                tt(Hi3, Q3, X2, MAX)

                # ---- horizontal merge (width 256 of 258) ----
                lo = [Lo3[:, :, d:d + W] for d in range(3)]
                me = [Me3[:, :, d:d + W] for d in range(3)]
                hi = [Hi3[:, :, d:d + W] for d in range(3)]

                def mtile(name):
                    t = m_pool.tile([128, R * W], f32, name=name, tag=name)
                    return t.rearrange("p (r w) -> p r w", w=W)

                mA = mtile("mA")
                mC = mtile("mC")
                mB = mtile("mB")
                mT = mtile("mT")
                mU = mtile("mU")
                mV = mtile("mV")

                # A = max3(lo)
                tt(mT, lo[0], lo[1], MAX)
                tt(mA, mT, lo[2], MAX)
                # C = min3(hi)
                tt(mU, hi[0], hi[1], MIN)
                tt(mC, mU, hi[2], MIN)
                # B = med3(me) = max(min(a,b), min(max(a,b), c))
                tt(mT, me[0], me[1], MIN)
                tt(mU, me[0], me[1], MAX)
                tt(mV, mU, me[2], MIN)
                tt(mB, mT, mV, MAX)

                # out = med3(A, B, C)
                ot = out_pool.tile([128, R * W], f32, name="ot", tag="ot")
                ot3 = ot.rearrange("p (r w) -> p r w", w=W)
                tt(mT, mA, mB, MIN)
                tt(mU, mA, mB, MAX)
                tt(mV, mU, mC, MIN)
                tt(ot3, mT, mV, MAX)

                # ---- store ----
                nc.sync.dma_start(out=og[:, r0:r0 + R, :], in_=ot3[0:64])
                nc.sync.dma_start(
                    out=og[:, HH + r0:HH + r0 + R, :], in_=ot3[64:128]
                )

    nc.compile()
    return nc


def _build_shared(R=8, gp_frac=0.0, dtype="float32", in_bufs=None, out_bufs=None):
    """15-op/pixel variant: vertical pair sharing + horizontal even/odd
    pair sharing in the merge.  gp_frac: fraction of rows of every
    elementwise op executed on GPSIMD instead of the DVE (unsupported by
    the current toolchain - keep 0).  dtype: compute dtype on-chip;
    float16 doubles DVE throughput on step-1 ops at ~2e-4 max rel err."""
    import concourse.bacc as bacc
    import concourse.mybir as mybir
    from concourse.tile import TileContext

    MIN = mybir.AluOpType.min
    MAX = mybir.AluOpType.max
    f32 = mybir.dt.float32
    cdt = getattr(mybir.dt, dtype)
    cast = cdt != f32

    assert HH % R == 0 and R % 2 == 0
    K = HH // R
    Rh = R // 2

    nc = bacc.Bacc("TRN2", name="median_pool2d_s")
    x = nc.dram_tensor("x", [C, H, W], f32, kind="ExternalInput")
    out = nc.dram_tensor("out", [C, H, W], f32, kind="ExternalOutput")
    xg = x.ap()
    og = out.ap()
    dma_io = nc.gpsimd if cast else nc.sync

    def tt(out_ap, in0, in1, op):
        rows = out_ap.shape[1]
        gp = int(rows * gp_frac + 0.5)
        split = rows - gp
        if split > 0:
            nc.vector.tensor_tensor(
                out=out_ap[:, 0:split], in0=in0[:, 0:split],
                in1=in1[:, 0:split], op=op,
            )
        if split < rows:
            nc.gpsimd.tensor_tensor(
                out=out_ap[:, split:rows], in0=in0[:, split:rows],
                in1=in1[:, split:rows], op=op,
            )

    if in_bufs is None:
        in_bufs = 3 if R <= 8 else 2
    if out_bufs is None:
        out_bufs = 3 if R <= 8 else 1
    with TileContext(nc) as tc:
        with (
            tc.tile_pool(name="io_in", bufs=in_bufs) as in_pool,
            tc.tile_pool(name="io_out", bufs=out_bufs) as out_pool,
            tc.tile_pool(name="work", bufs=1) as w_pool,
        ):
            def wtile(name, rows, width, tag=None):
                t = w_pool.tile([128, rows * width], cdt, name=name,
                                tag=tag or name)
                return t.rearrange("p (r w) -> p r w", w=width)

            for k in range(K):
                r0 = k * R
                it = in_pool.tile([128, (R + 2) * WP], cdt, name="it", tag="it")
                it3 = it.rearrange("p (r w) -> p r w", w=WP)
                nc.vector.memset(it3[:, :, 0:WP:WP - 1], 0.0)
                if k == 0:
                    nc.vector.memset(it3[0:64, 0:1, 1:W + 1], 0.0)
                    dma_io.dma_start(out=it3[0:64, 1:R + 2, 1:W + 1],
                                      in_=xg[:, 0:R + 1, :])
                else:
                    dma_io.dma_start(out=it3[0:64, :, 1:W + 1],
                                      in_=xg[:, r0 - 1:r0 + R + 1, :])
                if k == K - 1:
                    nc.vector.memset(it3[64:128, R + 1:R + 2, 1:W + 1], 0.0)
                    dma_io.dma_start(out=it3[64:128, 0:R + 1, 1:W + 1],
                                      in_=xg[:, HH + r0 - 1:H, :])
                else:
                    dma_io.dma_start(out=it3[64:128, :, 1:W + 1],
                                      in_=xg[:, HH + r0 - 1:HH + r0 + R + 1, :])

                # ---- vertical: shared pair sort ----
                # pairs over in-tile row pairs (2i+1, 2i+2), i = 0..R/2-1
                Pm = wtile("Pm", Rh, WP)
                PM = wtile("PM", Rh, WP)
                tt(Pm, it3[:, 1:R + 1:2, :], it3[:, 2:R + 2:2, :], MIN)
                tt(PM, it3[:, 1:R + 1:2, :], it3[:, 2:R + 2:2, :], MAX)

                Lo3 = wtile("Lo", R, WP)
                Me3 = wtile("Me", R, WP)
                Hi3 = wtile("Hi", R, WP)
                tE = wtile("tE", Rh, WP)
                tO = wtile("tO", Rh, WP)
                a_e = it3[:, 0:R:2, :]          # third element, even out rows
                a_o = it3[:, 3:R + 2:2, :]      # rows 3,5,..,R+1 (count R/2)
                # even out rows y=0,2,..  (pair index i=y/2)
                tt(Lo3[:, 0:R:2], a_e, Pm, MIN)
                tt(Hi3[:, 0:R:2], a_e, PM, MAX)
                tt(tE, a_e, PM, MIN)
                tt(Me3[:, 0:R:2], Pm, tE, MAX)
                # odd out rows y=1,3,..   (pair index i=(y-1)/2)
                tt(Lo3[:, 1:R:2], a_o, Pm, MIN)
                tt(Hi3[:, 1:R:2], a_o, PM, MAX)
                tt(tO, a_o, PM, MIN)
                tt(Me3[:, 1:R:2], Pm, tO, MAX)

                # ---- merge: horizontal shared pairs ----
                NP = W // 2 + 1                 # 129 pairs over padded width
                # Pm/PM/tE/tO are dead after the vertical completions;
                # alias their slots (Rh*WP = 2064 >= R*NP = 2064 elems).
                PA = wtile("PA", R, NP, tag="Pm")
                PC = wtile("PC", R, NP, tag="PM")
                Um = wtile("Um", R, NP, tag="tE")
                Vm = wtile("Vm", R, NP, tag="tO")
                # PA/PC (in Pm/PM slots) are dead once mA/mC are built;
                # rotate tBe/tBo through the same slots.
                tBe = wtile("tBe", R, W // 2, tag="Pm")
                tBo = wtile("tBo", R, W // 2, tag="PM")
                mA = wtile("mA", R, W)
                mB = wtile("mB", R, W)
                mC = wtile("mC", R, W)

                ev = slice(0, WP, 2)            # padded even cols (129)
                od = slice(1, WP, 2)            # padded odd cols (129)
                tt(PA, Lo3[:, :, ev], Lo3[:, :, od], MAX)
                tt(mA[:, :, 0:W:2], PA[:, :, 0:NP - 1], Lo3[:, :, 2:WP:2], MAX)
                tt(mA[:, :, 1:W:2], PA[:, :, 1:NP], Lo3[:, :, 1:WP - 2:2], MAX)

                tt(PC, Hi3[:, :, ev], Hi3[:, :, od], MIN)
                tt(mC[:, :, 0:W:2], PC[:, :, 0:NP - 1], Hi3[:, :, 2:WP:2], MIN)
                tt(mC[:, :, 1:W:2], PC[:, :, 1:NP], Hi3[:, :, 1:WP - 2:2], MIN)

                tt(Um, Me3[:, :, ev], Me3[:, :, od], MIN)
                tt(Vm, Me3[:, :, ev], Me3[:, :, od], MAX)
                tt(tBe, Me3[:, :, 2:WP:2], Vm[:, :, 0:NP - 1], MIN)
                tt(mB[:, :, 0:W:2], Um[:, :, 0:NP - 1], tBe, MAX)
                tt(tBo, Me3[:, :, 1:WP - 2:2], Vm[:, :, 1:NP], MIN)
                tt(mB[:, :, 1:W:2], Um[:, :, 1:NP], tBo, MAX)

                # ---- final med3(A, B, C) ----
                # Lo/Me/Hi are dead once the merge pairs+completions ran;
                # alias their slots (R*WP >= R*W).
                mT = wtile("mT", R, W, tag="Lo")
                mU = wtile("mU", R, W, tag="Me")
                mV = wtile("mV", R, W, tag="Hi")
                ot = out_pool.tile([128, R * W], cdt, name="ot", tag="ot")
                ot3 = ot.rearrange("p (r w) -> p r w", w=W)
                tt(mT, mA, mB, MIN)
                tt(mU, mA, mB, MAX)
                tt(mV, mU, mC, MIN)
                tt(ot3, mT, mV, MAX)

                dma_io.dma_start(out=og[:, r0:r0 + R, :], in_=ot3[0:64])
                dma_io.dma_start(out=og[:, HH + r0:HH + r0 + R, :],
                                  in_=ot3[64:128])

    nc.compile()
    return nc


def _build_copy():
    """Calibration kernel: pure DMA passthrough x -> out."""
    import concourse.bacc as bacc
    import concourse.mybir as mybir
    from concourse.tile import TileContext

    f32 = mybir.dt.float32
    nc = bacc.Bacc("TRN2", name="median_copy_cal")
    x = nc.dram_tensor("x", [C, H, W], f32, kind="ExternalInput")
    out = nc.dram_tensor("out", [C, H, W], f32, kind="ExternalOutput")
    xf = x.ap().rearrange("c h w -> (c h) w").rearrange(
        "(n p) w -> n p w", p=128)
    of = out.ap().rearrange("c h w -> (c h) w").rearrange(
        "(n p) w -> n p w", p=128)
    n = xf.shape[0]
    with TileContext(nc) as tc:
        with tc.tile_pool(name="io", bufs=4) as pool:
            for i in range(0, n, 8):
                t = pool.tile([128, 8 * W], f32, name="t", tag="t")
                t3 = t.rearrange("p (n w) -> p n w", w=W)
                nc.sync.dma_start(out=t3[:], in_=xf[i:i + 8].rearrange(
                    "n p w -> p n w"))
                nc.sync.dma_start(out=of[i:i + 8].rearrange("n p w -> p n w"),
                                  in_=t3[:])
    nc.compile()
    return nc


def _get_nc(R=8, gp_rows=0, shared=False, gp_frac=0.0, copy=False,
            dtype="float32", in_bufs=None, out_bufs=None):
    key = (R, gp_rows, shared, gp_frac, copy, dtype, in_bufs, out_bufs)
    if key not in _CACHE:
        if copy:
            _CACHE[key] = _build_copy()
        elif shared:
            _CACHE[key] = _build_shared(R=R, gp_frac=gp_frac, dtype=dtype,
                                        in_bufs=in_bufs, out_bufs=out_bufs)
        else:
            _CACHE[key] = _build(R=R, gp_rows=gp_rows)
    return _CACHE[key]


def kernel(x: np.ndarray) -> np.ndarray:
    """MedianPool2d(3x3, s=1, p=1) on 8 NeuronCores. Bit-exact vs fp32
    reference (pure min/max selection network, no arithmetic)."""
    from concourse.bass_utils import run_bass_kernel_spmd

    assert x.shape == (B, C, H, W), x.shape
    x = np.ascontiguousarray(x, dtype=np.float32)
    try:
        nc = _get_nc(shared=True, R=16)
    except Exception:
        # fall back to the simpler 18-op builder (also bit-exact)
        nc = _get_nc(R=8)
    in_maps = [{"x": x[i]} for i in range(NCORES)]
    res = run_bass_kernel_spmd(nc, in_maps, core_ids=list(range(NCORES)))
    return np.stack([r["out"] for r in res.results], axis=0)



# revision 2
# speedup vs baseline: 1.8954x; 1.8954x over previous
"""MedianPool2d (3x3, stride 1, zero-pad 1) Trainium2 Bass kernel.

Full input x: (8, 64, 256, 256) fp32.  Sharding: pure data parallel over
batch -> core i processes x[i] (64, 256, 256).

Per-core layout: 128 SBUF partitions = (h, c) with h in {0,1} picking the
top/bottom 128-row half of the image and c the channel.  Rows are stored
in fp16, column-DEINTERLEAVED: row = [E | O] where E[t] = padded col 2t
(129 wide, E[0] is the left zero-pad col) and O[t] = padded col 2t+1
(129 wide, O[128] is the right zero-pad col).

Median of 9 = med3(max3(column lo), med3(column me), min3(column hi)).
The deinterleaved layout makes every horizontal even/odd pair op a
contiguous-block op, so ALL DVE tensor_tensor instructions are fp16 with
unit-stride last dims -> they hit the DVE 2x_1p performance mode
(2 elem/cycle/lane).  Elems/px on DVE: vertical 5.04 (shared row pairs)
+ max3 1.50 + min3 1.50 + med3 3.01 + final med3 4 = 15.06 -> 7.53
cycles/px, ~2x faster than the fp32 15 op/px variant.

The Activation engine does fp32->fp16 cast + deinterleave on load and
fp16->fp32 cast + re-interleave on store (it cannot do two-tensor
min/max, but casts/copies run there for free alongside the DVE).
GPSIMD handles zero-pad memsets.  The neuron backend rejects
TensorTensor/TensorScalarPtr on GPSIMD, so the DVE does all min/max.

fp16 keeps the min/max selection network exact up to input rounding:
max rel err ~9.7e-4, l2 rel err ~2e-4 (gate is 2e-2).

Chunks of R=16 rows pipeline DMA -> ACT -> DVE -> ACT -> DMA; the first/
last chunks are tapered small to shrink pipeline fill/drain.
"""

import numpy as np

B, C, H, W = 8, 64, 256, 256
NCORES = 8
HH = H // 2           # rows per half-strip
WP = W + 2            # 258 padded width
NE = WP // 2          # 129 = evens block width (incl. left zero col)

_CACHE = {}


def _build_f16(R=16, taper=(2, 6, 16, 16, 16, 16, 16, 16, 16, 6, 2)):
    import concourse.bacc as bacc
    import concourse.mybir as mybir
    from concourse.tile import TileContext

    MIN = mybir.AluOpType.min
    MAX = mybir.AluOpType.max
    f32 = mybir.dt.float32
    f16 = mybir.dt.float16

    if isinstance(taper, (list, tuple)):
        chunks = list(taper)
    else:
        chunks = [R] * (HH // R)
    assert sum(chunks) == HH and all(c % 2 == 0 for c in chunks)
    K = len(chunks)

    nc = bacc.Bacc("TRN2", name="median_pool2d_f16")
    x = nc.dram_tensor("x", [C, H, W], f32, kind="ExternalInput")
    out = nc.dram_tensor("out", [C, H, W], f32, kind="ExternalOutput")
    xg = x.ap()
    og = out.ap()

    with TileContext(nc) as tc:
        with (
            tc.tile_pool(name="io_in", bufs=3) as in_pool,
            tc.tile_pool(name="x16p", bufs=2) as x16_pool,
            tc.tile_pool(name="work", bufs=1) as w_pool,
            tc.tile_pool(name="otp", bufs=2) as ot_pool,
            tc.tile_pool(name="io_out", bufs=2) as out_pool,
        ):
            x16rs = {}
            r0s = [sum(chunks[:i]) for i in range(K)]

            def load_dei(k):
                """DMA fp32 chunk k + ACT cast/deinterleave into x16."""
                r0, R = r0s[k], chunks[k]
                x32 = in_pool.tile([128, (R + 2) * W], f32, name="x32",
                                   tag="x32")
                x32r = x32.rearrange("p (r w) -> p r w", w=W)
                if k == 0:
                    nc.sync.dma_start(out=x32r[0:64, 1:R + 2, :],
                                      in_=xg[:, 0:R + 1, :])
                else:
                    nc.sync.dma_start(out=x32r[0:64, :, :],
                                      in_=xg[:, r0 - 1:r0 + R + 1, :])
                if k == K - 1:
                    nc.sync.dma_start(out=x32r[64:128, 0:R + 1, :],
                                      in_=xg[:, HH + r0 - 1:H, :])
                else:
                    nc.sync.dma_start(out=x32r[64:128, :, :],
                                      in_=xg[:, HH + r0 - 1:HH + r0 + R + 1, :])

                x16 = x16_pool.tile([128, (R + 2) * WP], f16, name="x16",
                                    tag="x16")
                x16r = x16.rearrange("p (r w) -> p r w", w=WP)

                def dei(pslice, rows):
                    # E: orig odd cols -> x16[,,1:129]; O: orig even -> 129:257
                    nc.scalar.copy(out=x16r[pslice, rows, 1:NE],
                                   in_=x32r[pslice, rows, 1:W:2])
                    nc.scalar.copy(out=x16r[pslice, rows, NE:WP - 1],
                                   in_=x32r[pslice, rows, 0:W:2])

                if k == 0:
                    dei(slice(0, 64), slice(1, R + 2))
                    dei(slice(64, 128), slice(0, R + 2))
                    nc.gpsimd.memset(x16r[0:64, 0:1, :], 0.0)
                elif k == K - 1:
                    dei(slice(0, 64), slice(0, R + 2))
                    dei(slice(64, 128), slice(0, R + 1))
                    nc.gpsimd.memset(x16r[64:128, R + 1:R + 2, :], 0.0)
                else:
                    dei(slice(0, 128), slice(0, R + 2))
                # zero pad columns E[0], O[128] for all rows
                nc.gpsimd.memset(x16r[:, :, 0:WP:WP - 1], 0.0)
                x16rs[k] = x16r

            load_dei(0)
            for k in range(K):
                r0, R = r0s[k], chunks[k]
                Rh = R // 2
                # prefetch + deinterleave next chunk BEFORE this chunk's
                # out-interleave so ACT never stalls the next DVE chunk
                if k + 1 < K:
                    load_dei(k + 1)
                x16r = x16rs.pop(k)

                # ---------- vertical sort3 (shared row pairs) ----------
                def wt(name, rows, width, tag=None):
                    t = w_pool.tile([128, rows * width], f16, name=name,
                                    tag=tag or name)
                    return t.rearrange("p (r w) -> p r w", w=width)

                Pm = wt("Pm", Rh, WP)
                PM = wt("PM", Rh, WP)
                tE = wt("tE", Rh, WP)
                tO = wt("tO", Rh, WP)
                Lo = wt("Lo", R, WP)
                Me = wt("Me", R, WP)
                Hi = wt("Hi", R, WP)

                b0 = x16r[:, 1:R + 1:2, :]
                b1 = x16r[:, 2:R + 2:2, :]
                a_e = x16r[:, 0:R:2, :]
                a_o = x16r[:, 3:R + 2:2, :]
                TT = nc.vector.tensor_tensor
                TT(out=Pm[:], in0=b0, in1=b1, op=MIN)
                TT(out=PM[:], in0=b0, in1=b1, op=MAX)
                TT(out=Lo[:, 0:R:2], in0=a_e, in1=Pm[:], op=MIN)
                TT(out=Hi[:, 0:R:2], in0=a_e, in1=PM[:], op=MAX)
                TT(out=tE[:], in0=a_e, in1=PM[:], op=MIN)
                TT(out=Me[:, 0:R:2], in0=Pm[:], in1=tE[:], op=MAX)
                TT(out=Lo[:, 1:R:2], in0=a_o, in1=Pm[:], op=MIN)
                TT(out=Hi[:, 1:R:2], in0=a_o, in1=PM[:], op=MAX)
                TT(out=tO[:], in0=a_o, in1=PM[:], op=MIN)
                TT(out=Me[:, 1:R:2], in0=Pm[:], in1=tO[:], op=MAX)

                # ---------- horizontal merge on deinterleaved blocks ------
                # pair tiles are Rh*WP = R*NE elems: alias dead V slots
                PA = wt("PA", R, NE, tag="Pm")
                PC = wt("PC", R, NE, tag="PM")
                u = wt("u", R, NE, tag="tE")
                v = wt("v", R, NE, tag="tO")
                A = wt("A", R, W)
                Bt = wt("Bt", R, W)
                Ct = wt("Ct", R, W)

                # A = max3(Lo): P = max(E,O); even: max(P[t], E[t+1]);
                # odd: max(O[t], P[t+1])
                TT(out=PA[:], in0=Lo[:, :, 0:NE], in1=Lo[:, :, NE:WP], op=MAX)
                TT(out=A[:, :, 0:128], in0=PA[:, :, 0:128],
                   in1=Lo[:, :, 1:NE], op=MAX)
                TT(out=A[:, :, 128:256], in0=Lo[:, :, NE:WP - 1],
                   in1=PA[:, :, 1:NE], op=MAX)
                # C = min3(Hi)
                TT(out=PC[:], in0=Hi[:, :, 0:NE], in1=Hi[:, :, NE:WP], op=MIN)
                TT(out=Ct[:, :, 0:128], in0=PC[:, :, 0:128],
                   in1=Hi[:, :, 1:NE], op=MIN)
                TT(out=Ct[:, :, 128:256], in0=Hi[:, :, NE:WP - 1],
                   in1=PC[:, :, 1:NE], op=MIN)
                # B = med3(Me): u = min(E,O), v = max(E,O);
                # even: max(u[t], min(v[t], E[t+1]));
                # odd:  max(min(O[t], v[t+1]), u[t+1])
                TT(out=u[:], in0=Me[:, :, 0:NE], in1=Me[:, :, NE:WP], op=MIN)
                TT(out=v[:], in0=Me[:, :, 0:NE], in1=Me[:, :, NE:WP], op=MAX)
                t1 = wt("t1", R, 128, tag="Pm")   # PA dead once A built
                TT(out=t1[:], in0=v[:, :, 0:128], in1=Me[:, :, 1:NE], op=MIN)
                TT(out=Bt[:, :, 0:128], in0=u[:, :, 0:128], in1=t1[:], op=MAX)
                t2 = wt("t2", R, 128, tag="PM")   # PC dead once C built
                TT(out=t2[:], in0=Me[:, :, NE:WP - 1], in1=v[:, :, 1:NE],
                   op=MIN)
                TT(out=Bt[:, :, 128:256], in0=t2[:], in1=u[:, :, 1:NE],
                   op=MAX)

                # ---------- final med3(A, B, C) ----------
                mT = wt("mT", R, W, tag="Lo")
                mU = wt("mU", R, W, tag="Me")
                mV = wt("mV", R, W, tag="Hi")
                ot = ot_pool.tile([128, R * W], f16, name="ot", tag="ot")
                otr = ot.rearrange("p (r w) -> p r w", w=W)
                TT(out=mT[:], in0=A[:], in1=Bt[:], op=MIN)
                TT(out=mU[:], in0=A[:], in1=Bt[:], op=MAX)
                TT(out=mV[:], in0=mU[:], in1=Ct[:], op=MIN)
                TT(out=otr[:], in0=mT[:], in1=mV[:], op=MAX)

                # ---------- re-interleave + cast to fp32 on ACT ----------
                o32 = out_pool.tile([128, R * W], f32, name="o32", tag="o32")
                o32r = o32.rearrange("p (r w) -> p r w", w=W)
                nc.scalar.copy(out=o32r[:, :, 0:W:2], in_=otr[:, :, 0:128])
                nc.scalar.copy(out=o32r[:, :, 1:W:2], in_=otr[:, :, 128:256])

                # ---------- store ----------
                nc.sync.dma_start(out=og[:, r0:r0 + R, :], in_=o32r[0:64])
                nc.sync.dma_start(out=og[:, HH + r0:HH + r0 + R, :],
                                  in_=o32r[64:128])

    nc.compile()
    return nc


def _build_shared(R=16, dtype="float32"):
    """Fallback: fp32 15-op/pixel variant (vertical pair sharing +
    strided horizontal even/odd sharing).  Bit-exact vs the reference."""
    import concourse.bacc as bacc
    import concourse.mybir as mybir
    from concourse.tile import TileContext

    MIN = mybir.AluOpType.min
    MAX = mybir.AluOpType.max
    f32 = mybir.dt.float32

    assert HH % R == 0 and R % 2 == 0
    K = HH // R
    Rh = R // 2

    nc = bacc.Bacc("TRN2", name="median_pool2d_s")
    x = nc.dram_tensor("x", [C, H, W], f32, kind="ExternalInput")
    out = nc.dram_tensor("out", [C, H, W], f32, kind="ExternalOutput")
    xg = x.ap()
    og = out.ap()

    def tt(out_ap, in0, in1, op):
        nc.vector.tensor_tensor(out=out_ap, in0=in0, in1=in1, op=op)

    with TileContext(nc) as tc:
        with (
            tc.tile_pool(name="io_in", bufs=2) as in_pool,
            tc.tile_pool(name="io_out", bufs=1) as out_pool,
            tc.tile_pool(name="work", bufs=1) as w_pool,
        ):
            def wtile(name, rows, width, tag=None):
                t = w_pool.tile([128, rows * width], f32, name=name,
                                tag=tag or name)
                return t.rearrange("p (r w) -> p r w", w=width)

            for k in range(K):
                r0 = k * R
                it = in_pool.tile([128, (R + 2) * WP], f32, name="it",
                                  tag="it")
                it3 = it.rearrange("p (r w) -> p r w", w=WP)
                nc.vector.memset(it3[:, :, 0:WP:WP - 1], 0.0)
                if k == 0:
                    nc.vector.memset(it3[0:64, 0:1, 1:W + 1], 0.0)
                    nc.sync.dma_start(out=it3[0:64, 1:R + 2, 1:W + 1],
                                      in_=xg[:, 0:R + 1, :])
                else:
                    nc.sync.dma_start(out=it3[0:64, :, 1:W + 1],
                                      in_=xg[:, r0 - 1:r0 + R + 1, :])
                if k == K - 1:
                    nc.vector.memset(it3[64:128, R + 1:R + 2, 1:W + 1], 0.0)
                    nc.sync.dma_start(out=it3[64:128, 0:R + 1, 1:W + 1],
                                      in_=xg[:, HH + r0 - 1:H, :])
                else:
                    nc.sync.dma_start(out=it3[64:128, :, 1:W + 1],
                                      in_=xg[:, HH + r0 - 1:HH + r0 + R + 1, :])

                Pm = wtile("Pm", Rh, WP)
                PM = wtile("PM", Rh, WP)
                tt(Pm[:], it3[:, 1:R + 1:2, :], it3[:, 2:R + 2:2, :], MIN)
                tt(PM[:], it3[:, 1:R + 1:2, :], it3[:, 2:R + 2:2, :], MAX)

                Lo3 = wtile("Lo", R, WP)
                Me3 = wtile("Me", R, WP)
                Hi3 = wtile("Hi", R, WP)
                tEv = wtile("tEv", Rh, WP)
                tOv = wtile("tOv", Rh, WP)
                a_e = it3[:, 0:R:2, :]
                a_o = it3[:, 3:R + 2:2, :]
                tt(Lo3[:, 0:R:2], a_e, Pm[:], MIN)
                tt(Hi3[:, 0:R:2], a_e, PM[:], MAX)
                tt(tEv[:], a_e, PM[:], MIN)
                tt(Me3[:, 0:R:2], Pm[:], tEv[:], MAX)
                tt(Lo3[:, 1:R:2], a_o, Pm[:], MIN)
                tt(Hi3[:, 1:R:2], a_o, PM[:], MAX)
                tt(tOv[:], a_o, PM[:], MIN)
                tt(Me3[:, 1:R:2], Pm[:], tOv[:], MAX)

                NP = W // 2 + 1
                PA = wtile("PA", R, NP, tag="Pm")
                PC = wtile("PC", R, NP, tag="PM")
                Um = wtile("Um", R, NP, tag="tEv")
                Vm = wtile("Vm", R, NP, tag="tOv")
                tBe = wtile("tBe", R, W // 2, tag="Pm")
                tBo = wtile("tBo", R, W // 2, tag="PM")
                mA = wtile("mA", R, W)
                mB = wtile("mB", R, W)
                mC = wtile("mC", R, W)

                ev = slice(0, WP, 2)
                od = slice(1, WP, 2)
                tt(PA[:], Lo3[:, :, ev], Lo3[:, :, od], MAX)
                tt(mA[:, :, 0:W:2], PA[:, :, 0:NP - 1], Lo3[:, :, 2:WP:2], MAX)
                tt(mA[:, :, 1:W:2], PA[:, :, 1:NP], Lo3[:, :, 1:WP - 2:2], MAX)

                tt(PC[:], Hi3[:, :, ev], Hi3[:, :, od], MIN)
                tt(mC[:, :, 0:W:2], PC[:, :, 0:NP - 1], Hi3[:, :, 2:WP:2], MIN)
                tt(mC[:, :, 1:W:2], PC[:, :, 1:NP], Hi3[:, :, 1:WP - 2:2], MIN)

                tt(Um[:], Me3[:, :, ev], Me3[:, :, od], MIN)
                tt(Vm[:], Me3[:, :, ev], Me3[:, :, od], MAX)
                tt(tBe[:], Me3[:, :, 2:WP:2], Vm[:, :, 0:NP - 1], MIN)
                tt(mB[:, :, 0:W:2], Um[:, :, 0:NP - 1], tBe[:], MAX)
                tt(tBo[:], Me3[:, :, 1:WP - 2:2], Vm[:, :, 1:NP], MIN)
                tt(mB[:, :, 1:W:2], Um[:, :, 1:NP], tBo[:], MAX)

                mT = wtile("mT", R, W, tag="Lo")
                mU = wtile("mU", R, W, tag="Me")
                mV = wtile("mV", R, W, tag="Hi")
                ot = out_pool.tile([128, R * W], f32, name="ot", tag="ot")
                ot3 = ot.rearrange("p (r w) -> p r w", w=W)
                tt(mT[:], mA[:], mB[:], MIN)
                tt(mU[:], mA[:], mB[:], MAX)
                tt(mV[:], mU[:], mC[:], MIN)
                tt(ot3[:], mT[:], mV[:], MAX)

                nc.sync.dma_start(out=og[:, r0:r0 + R, :], in_=ot3[0:64])
                nc.sync.dma_start(out=og[:, HH + r0:HH + r0 + R, :],
                                  in_=ot3[64:128])

    nc.compile()
    return nc


def _get_nc(kind="f16", **kw):
    key = (kind, tuple(sorted(kw.items())))
    if key not in _CACHE:
        if kind == "f16":
            _CACHE[key] = _build_f16(**kw)
        else:
            _CACHE[key] = _build_shared(**kw)
    return _CACHE[key]


def kernel(x: np.ndarray) -> np.ndarray:
    """MedianPool2d(3x3, s=1, p=1) on 8 NeuronCores, data parallel over
    batch.  fp16 min/max selection network (exact up to fp16 input
    rounding, l2 rel err ~2e-4)."""
    from concourse.bass_utils import run_bass_kernel_spmd

    assert x.shape == (B, C, H, W), x.shape
    x = np.ascontiguousarray(x, dtype=np.float32)
    try:
        nc = _get_nc("f16")
    except Exception:
        # fall back to the fp32 builder (bit-exact)
        nc = _get_nc("f32", R=16)
    in_maps = [{"x": x[i]} for i in range(NCORES)]
    res = run_bass_kernel_spmd(nc, in_maps, core_ids=list(range(NCORES)))
    return np.stack([r["out"] for r in res.results], axis=0)
